# revision 15
# baseline (speedup 1.0000x reference)
"""Trainium2 Bass kernel for BERelativeSelfMultiheadAttn.

Strategy (data-parallel over batch B=8, one batch per NeuronCore):
  - Host folds the BatchEnsemble scale vectors r_*/s_* and the attention
    scale 1/sqrt(hd) into per-batch effective weight matrices (bf16),
    transposes x/pos, and lays weights out kb-major ([128, H] tiles) so
    every DMA descriptor is a contiguous 2KB row.
  - All weights/activations are prefetched into SBUF up-front: the TRN2
    tensor engine only reaches its top p-state after ~3us of continuous
    execution, so PE stream density is the primary performance lever.
  - Q/K/rk are computed transposed ([feature, t]); V in [t, feature]
    layout with an extra ones column that yields the softmax
    normalizer Z for free.
  - Relative shift via the flat-buffer trick: bd [T, Lr] rows are written
    to DRAM as [T, 1+Lr] bf16 (zero in col 0) and the shifted matrix is
    read back CONTIGUOUSLY as [q, k] tiles (the xbar-transpose DMA is
    descriptor-bound, and an f32 round trip saturates HBM).  Each
    [128,128] block is PE-transposed into a bf16 PSUM tile, staged to
    SBUF on the DVE, and accumulated into the AC score PSUM with an
    identity matmul; exp reads the PSUM directly.
  - The Z chain (ln/exp/ones-matmul broadcast/multiply) is deferred into
    the next head's stage stream, with its scalar and PE parts released
    at different points, so neither the scalar queue ahead of exp nor
    the PE ever waits on it.
"""

import numpy as np
import ml_dtypes

import concourse.bass as bass
import concourse.mybir as mybir
import concourse.tile as tile
from concourse import bacc
from concourse.bass_utils import run_bass_kernel_spmd

F32 = mybir.dt.float32
BF16 = mybir.dt.bfloat16
Act = mybir.ActivationFunctionType

P = 128


def build_program(T=1024, H=1024, heads=16, num_devices=8, enable_asserts=False):
    hd = H // heads
    assert hd == 64, "layout assumes head dim 64"
    nT = T // P            # t/q/k/r 128-blocks
    nH = H // P            # hidden-feature 128-blocks
    hpb = P // hd          # heads per 128-block (2)
    npair = heads // hpb
    CHT = min(512, T)      # matmul N chunk along T
    nCT = T // CHT
    CHH = min(512, H)      # matmul N chunk along H (V features)
    nCH_ = H // CHH
    Lr = T

    nc = bacc.Bacc(
        "TRN2",
        target_bir_lowering=False,
        debug=False,
        enable_asserts=enable_asserts,
        num_devices=num_devices,
    )

    xT_d = nc.dram_tensor("xT", [H, T], BF16, kind="ExternalInput").ap()
    posT_d = nc.dram_tensor("posT", [H, Lr], BF16, kind="ExternalInput").ap()
    wq_d = nc.dram_tensor("wq", [nH, P, H], BF16, kind="ExternalInput").ap()
    wk_d = nc.dram_tensor("wk", [nH, P, H], BF16, kind="ExternalInput").ap()
    wp_d = nc.dram_tensor("wp", [nH, P, H], BF16, kind="ExternalInput").ap()
    wv_d = nc.dram_tensor("wv", [nH, P, H], BF16, kind="ExternalInput").ap()
    wo_d = nc.dram_tensor("wo", [npair, P, H], BF16, kind="ExternalInput").ap()
    bqrw_d = nc.dram_tensor("bqrw", [nH, P, 1], F32, kind="ExternalInput").ap()
    bk_d = nc.dram_tensor("bk", [nH, P, 1], F32, kind="ExternalInput").ap()
    bp_d = nc.dram_tensor("bp", [nH, P, 1], F32, kind="ExternalInput").ap()
    drr_d = nc.dram_tensor("drr", [nH, P, 1], F32, kind="ExternalInput").ap()
    bo_d = nc.dram_tensor("bo", [nH, P, 1], F32, kind="ExternalInput").ap()
    ident_d = nc.dram_tensor("ident", [P, P], BF16, kind="ExternalInput").ap()
    outT_d = nc.dram_tensor("outT", [H, T], F32, kind="ExternalOutput").ap()

    with tile.TileContext(nc) as tc:
        with (
            tc.tile_pool(name="const", bufs=1) as constp,
            tc.tile_pool(name="persist", bufs=1) as pp,
        ):
            # constants
            id_sb = constp.tile([P, P], BF16, tag="ident")
            nc.sync.dma_start(id_sb[:], ident_d[:])
            ones_sb = constp.tile([P, hd], BF16, tag="ones")
            nc.vector.memset(ones_sb[:], 1.0)
            bqrw_t, bk_t, bp_t, drr_t, bo_t = [], [], [], [], []
            for jo in range(nH):
                for lst, d, nm in (
                    (bqrw_t, bqrw_d, "bqrw"),
                    (bk_t, bk_d, "bk"),
                    (bp_t, bp_d, "bp"),
                    (drr_t, drr_d, "drr"),
                    (bo_t, bo_d, "bo"),
                ):
                    t = constp.tile([P, 1], F32, tag=f"{nm}{jo}", name=f"{nm}_{jo}")
                    nc.sync.dma_start(t[:], d[jo])
                    lst.append(t)

            # persistent activation tensors
            rkT = [pp.tile([P, Lr], BF16, tag=f"rk{i}", name=f"rkT{i}")
                   for i in range(nH)]
            Qrw = [pp.tile([P, T], BF16, tag=f"qrw{i}", name=f"Qrw{i}")
                   for i in range(nH)]
            Kt = [pp.tile([P, T], BF16, tag=f"kt{i}", name=f"Kt{i}")
                  for i in range(nH)]
            Vsb = [pp.tile([P, heads * (hd + 1)], BF16, tag=f"v{i}", name=f"Vsb{i}")
                   for i in range(nT)]
            # context, packed per head-pair: head 2p in partitions 0:64,
            # head 2p+1 in partitions 64:128
            ctxp = [pp.tile([P, T], BF16, tag=f"ctx{p}", name=f"ctxp{p}")
                    for p in range(npair)]
            # Qrr = Qrw + (rr - rw) bias, one per pair, staged early so the
            # bd blocks never wait on the scalar queue
            qrr_sb = [pp.tile([P, T], BF16, tag=f"qrr{p}", name=f"qrr{p}")
                      for p in range(npair)]

            with (
                tc.tile_pool(name="bdout", bufs=3) as bdoutp,
                tc.tile_pool(name="bdd", bufs=6, space=bass.MemorySpace.DRAM) as dramp,
            ):

                def emit_qrr(pr):
                    nc.scalar.activation(
                        qrr_sb[pr][:], Qrw[pr][:], Act.Identity,
                        bias=drr_t[pr][:])

                def emit_bd(pr, bdd, pool, tag):
                    """bd scores for pair pr in [q, r] layout -> DRAM flat."""
                    for qi in range(nT):
                        for h in range(hpb):
                            base = h * hd
                            psb = pool.tile([P, T], F32, tag=tag, name="psbd")
                            for c in range(nCT):
                                nc.tensor.matmul(
                                    psb[:, c * CHT:(c + 1) * CHT],
                                    qrr_sb[pr][base:base + hd,
                                               qi * P:(qi + 1) * P],
                                    rkT[pr][base:base + hd, c * CHT:(c + 1) * CHT],
                                    start=True, stop=True,
                                )
                            bdo = bdoutp.tile([P, T + 1], BF16, name="bdo")
                            nc.vector.memset(bdo[:, 0:1], 0.0)
                            nc.vector.tensor_copy(bdo[:, 1:T + 1], psb[:])
                            nc.sync.dma_start(
                                bdd[h][qi * P:(qi + 1) * P, :], bdo[:])

                # -------- Phases 1+2: rk, Q, K (transposed), V (direct) ----
                with (
                    tc.tile_pool(name="ps12", bufs=2,
                                 space=bass.MemorySpace.PSUM) as psp2,
                    tc.tile_pool(name="pos", bufs=1) as pospool,
                    tc.tile_pool(name="xp", bufs=1) as xpool,
                    tc.tile_pool(name="wqkp", bufs=1) as wpool,
                    tc.tile_pool(name="vtmp", bufs=2) as vtmpp,
                ):
                    posT_sb = [pospool.tile([P, Lr], BF16, tag=f"pos{i}",
                                            name=f"posT{i}") for i in range(nH)]
                    xT_sb = [xpool.tile([P, T], BF16, tag=f"x{i}", name=f"xT{i}")
                             for i in range(nH)]
                    w_sb = {}
                    for wname in ("wp", "wq", "wk", "wv"):
                        for kb in range(nH):
                            w_sb[(wname, kb)] = wpool.tile(
                                [P, H], BF16, tag=f"{wname}{kb}",
                                name=f"{wname}{kb}")

                    # DMA issue order = priority order
                    for kb in range(nH):
                        nc.sync.dma_start(
                            posT_sb[kb][:], posT_d[kb * P:(kb + 1) * P, :])
                        nc.sync.dma_start(w_sb[("wp", kb)][:], wp_d[kb])
                    for kb in range(nH):
                        nc.sync.dma_start(
                            xT_sb[kb][:], xT_d[kb * P:(kb + 1) * P, :])
                        nc.sync.dma_start(w_sb[("wq", kb)][:], wq_d[kb])
                        nc.sync.dma_start(w_sb[("wk", kb)][:], wk_d[kb])
                    for kb in range(nH):
                        nc.sync.dma_start(w_sb[("wv", kb)][:], wv_d[kb])

                    def proj_block(jo, wname, src_sb, bias_t, dst):
                        pss = psp2.tile([P, T], F32, tag="ps12", name="ps12t")
                        for kb in range(nH):
                            for c in range(nCT):
                                nc.tensor.matmul(
                                    pss[:, c * CHT:(c + 1) * CHT],
                                    w_sb[(wname, kb)][:, jo * P:(jo + 1) * P],
                                    src_sb[kb][:, c * CHT:(c + 1) * CHT],
                                    start=(kb == 0),
                                    stop=(kb == nH - 1),
                                )
                        nc.scalar.activation(
                            dst[jo][:], pss[:], Act.Identity, bias=bias_t[jo][:])

                    # jo=0 first, then bd(0) so its DRAM round trip overlaps
                    # the remaining projection work.
                    proj_block(0, "wp", posT_sb, bp_t, rkT)
                    proj_block(0, "wq", xT_sb, bqrw_t, Qrw)
                    proj_block(0, "wk", xT_sb, bk_t, Kt)
                    emit_qrr(0)
                    bdd_prev = [dramp.tile([T, T + 1], BF16, name="bddram")
                                for _ in range(hpb)]
                    emit_bd(0, bdd_prev, psp2, "ps12")

                    # V next so the GpSimd scatters finish before attention.
                    for ti in range(nT):
                        nc.vector.memset(Vsb[ti][:], 1.0)
                        psv = psp2.tile([P, H], F32, tag="ps12", name="psvt")
                        for kb in range(nH):
                            for c in range(nCH_):
                                nc.tensor.matmul(
                                    psv[:, c * CHH:(c + 1) * CHH],
                                    xT_sb[kb][:, ti * P:(ti + 1) * P],
                                    w_sb[("wv", kb)][:, c * CHH:(c + 1) * CHH],
                                    start=(kb == 0),
                                    stop=(kb == nH - 1),
                                )
                        vtmp = vtmpp.tile([P, H], BF16)
                        nc.vector.tensor_copy(vtmp[:], psv[:])
                        nc.gpsimd.tensor_copy(
                            Vsb[ti][:].rearrange(
                                "p (h e) -> p h e", e=hd + 1)[:, :, 0:hd],
                            vtmp[:].rearrange("p (h d) -> p h d", d=hd),
                        )

                    for jo in range(1, nH):
                        proj_block(jo, "wp", posT_sb, bp_t, rkT)
                        proj_block(jo, "wq", xT_sb, bqrw_t, Qrw)
                        proj_block(jo, "wk", xT_sb, bk_t, Kt)
                        emit_qrr(jo)
                        if jo == 1:
                            bdd_1 = [dramp.tile([T, T + 1], BF16,
                                                name="bddram")
                                     for _ in range(hpb)]
                            emit_bd(1, bdd_1, psp2, "ps12")

                # ---------------- Phase 3: per head-pair attention ---------
                with (
                    tc.tile_pool(name="psS", bufs=2,
                                 space=bass.MemorySpace.PSUM) as psS,
                    tc.tile_pool(name="bdsh", bufs=2) as bdshp,
                    tc.tile_pool(name="xstg", bufs=3) as xstgp,
                    tc.tile_pool(name="pT", bufs=3) as pTp,
                    tc.tile_pool(name="norm", bufs=2) as normp,
                    tc.tile_pool(name="psX", bufs=2, space=bass.MemorySpace.PSUM) as psX,
                    tc.tile_pool(name="psC", bufs=1, space=bass.MemorySpace.PSUM) as psC,
                ):
                    def base2(h):
                        return h * hd

                    # deferred normalize closures, split so the Z chain's
                    # scalar ops and its PE ops never stall the pipeline
                    pending_sc = []   # ln+zi (scalar)
                    pending_pe = []   # zb matmul + ctx multiply

                    def emit_readbacks(bdd):
                        bdsh = {}
                        for h in range(hpb):
                            flat = bdd[h][:].rearrange("a b -> (a b)")
                            bdview = flat[T:T + T * T].rearrange(
                                "(a b) -> a b", b=T)
                            for qi in range(nT):
                                t = bdshp.tile([P, T], BF16, tag=f"bdsh{qi}",
                                               name=f"bdsh{h}_{qi}")
                                nc.sync.dma_start(
                                    t[:], bdview[qi * P:(qi + 1) * P, :])
                                bdsh[(h, qi)] = t
                        return bdsh

                    def emit_attn(pr, bdsh, emit_bd_next):
                        stages = [(h, kb) for h in range(hpb)
                                  for kb in range(nT)]
                        pscs = {}
                        Ys = {}
                        xsbs = {}
                        pTs = {}

                        def emit_pv(i):
                            h, kb = stages[i]
                            habs = pr * hpb + h
                            for c in range(nCT):
                                cs = slice(c * CHT, (c + 1) * CHT)
                                nc.tensor.matmul(
                                    pscs[h][:, cs],
                                    Vsb[kb][:, habs * (hd + 1):
                                            (habs + 1) * (hd + 1)],
                                    pTs[i][:, cs],
                                    start=(kb == 0), stop=(kb == nT - 1),
                                )

                        def emit_idadd_exp(i):
                            # accumulate bd^T into the AC PSUM, then exp
                            for c in range(nCT):
                                cs = slice(c * CHT, (c + 1) * CHT)
                                nc.tensor.matmul(
                                    Ys[i][:, cs], id_sb[:], xsbs[i][:, cs],
                                    start=False, stop=True,
                                )
                            pT_t = pTp.tile([P, T], BF16, name="pT_t")
                            nc.scalar.activation(pT_t[:], Ys[i][:], Act.Exp)
                            pTs[i] = pT_t

                        def emit_norm_stage(h):
                            # normalize: ctx = psc[0:hd]/Z ; Z = psc[hd]
                            base = h * hd
                            cstage = normp.tile([hd + 1, T], BF16, tag="cs",
                                                name="cstage")
                            nc.vector.tensor_copy(cstage[:], pscs[h][:])
                            lnz = normp.tile([hd + 1, T], F32, tag="lnz",
                                             name="lnz")
                            zi = normp.tile([hd + 1, T], BF16, tag="zi",
                                            name="zi")

                            def finish_sc(cstage=cstage, lnz=lnz, zi=zi):
                                nc.scalar.activation(
                                    lnz[hd:hd + 1, :], cstage[hd:hd + 1, :],
                                    Act.Ln)
                                with nc.allow_low_precision(reason="zinv"):
                                    nc.scalar.activation(
                                        zi[hd:hd + 1, :], lnz[hd:hd + 1, :],
                                        Act.Exp, scale=-1.0)

                            def finish_pe(pr=pr, base=base, cstage=cstage,
                                          zi=zi):
                                zb = psS.tile([P, T], F32, tag="s", name="zb")
                                for c in range(nCT):
                                    cs = slice(c * CHT, (c + 1) * CHT)
                                    nc.tensor.matmul(
                                        zb[0:hd, cs],
                                        ones_sb[hd:hd + 1, :],
                                        zi[hd:hd + 1, cs],
                                        start=True, stop=True,
                                    )
                                with nc.allow_low_precision(reason="ctx"):
                                    nc.vector.tensor_mul(
                                        ctxp[pr][base:base + hd, :],
                                        cstage[0:hd, :], zb[0:hd, :])

                            pending_sc.append(finish_sc)
                            pending_pe.append(finish_pe)

                        for i, (h, kb) in enumerate(stages):
                            if kb == 0:
                                pscs[h] = psC.tile([hd + 1, T], F32,
                                                   name="psc")
                            if i == 10:
                                emit_norm_stage(0)
                            # PE: transpose shifted-bd blocks into PSUM
                            X = psX.tile([P, T], BF16, tag="x", name="psx")
                            for qi in range(nT):
                                nc.tensor.matmul(
                                    X[:, qi * P:(qi + 1) * P],
                                    bdsh[(h, qi)][:, kb * P:(kb + 1) * P],
                                    id_sb[:],
                                    is_transpose=True,
                                    start=True, stop=True,
                                )
                            # PE: AC scores [k, q]; group stays open for
                            # the identity-add
                            Y = psS.tile([P, T], F32, tag="s", name="psy")
                            for c in range(nCT):
                                cs = slice(c * CHT, (c + 1) * CHT)
                                nc.tensor.matmul(
                                    Y[:, cs],
                                    Kt[pr][base2(h):base2(h) + hd,
                                           kb * P:(kb + 1) * P],
                                    Qrw[pr][base2(h):base2(h) + hd, cs],
                                    start=True, stop=False,
                                )
                            Ys[i] = Y
                            # stage X -> SBUF (DVE)
                            xsb = xstgp.tile([P, T], BF16, name="x_sb")
                            nc.vector.tensor_copy(xsb[:], X[:])
                            xsbs[i] = xsb
                            if i == 3 and pending_sc:
                                pending_sc.pop(0)()
                            if i == 5 and pending_pe:
                                pending_pe.pop(0)()
                            if i == 11 and pending_sc:
                                pending_sc.pop(0)()
                            if i == 13 and pending_pe:
                                pending_pe.pop(0)()
                            if i >= 1:
                                emit_idadd_exp(i - 1)
                            if i >= 2:
                                emit_pv(i - 2)
                        # next pair's bd block runs here so the DVE/scalar
                        # queues drain under PE cover before the tail
                        if emit_bd_next is not None:
                            emit_bd_next()
                        emit_idadd_exp(15)
                        emit_pv(14)
                        emit_pv(15)
                        emit_norm_stage(1)

                    bdds = {0: bdd_prev, 1: bdd_1}

                    def make_bd_next(pr2):
                        def f():
                            bdds[pr2] = [dramp.tile([T, T + 1], BF16,
                                                    name="bddram")
                                         for _ in range(hpb)]
                            emit_bd(pr2, bdds[pr2], psS, "s")
                        return f

                    for pr in range(npair):
                        bdsh = emit_readbacks(bdds[pr])
                        nxt = make_bd_next(pr + 2) if pr + 2 < npair else None
                        emit_attn(pr, bdsh, nxt)

                    # ------------ Phase 4: output projection ---------------
                    # jo=0's accumulation is split around the final Z chain
                    # so the PE keeps running while it completes.
                    with (
                        tc.tile_pool(name="wop", bufs=1) as wop,
                        tc.tile_pool(name="outb", bufs=3) as outp,
                    ):
                        wo_sb = [wop.tile([P, H], BF16, tag=f"wo{p}",
                                          name=f"wo{p}")
                                 for p in range(npair)]
                        for p in range(npair):
                            nc.sync.dma_start(wo_sb[p][:], wo_d[p])

                        def ph4_mm(pss, jo, p):
                            for c in range(nCT):
                                nc.tensor.matmul(
                                    pss[:, c * CHT:(c + 1) * CHT],
                                    wo_sb[p][:, jo * P:(jo + 1) * P],
                                    ctxp[p][:, c * CHT:(c + 1) * CHT],
                                    start=(p == 0),
                                    stop=(p == npair - 1),
                                )

                        def ph4_tail(pss, jo):
                            ot = outp.tile([P, T], F32)
                            nc.scalar.activation(
                                ot[:], pss[:], Act.Identity, bias=bo_t[jo][:])
                            nc.sync.dma_start(
                                outT_d[jo * P:(jo + 1) * P, :], ot[:])

                        pss0 = psS.tile([P, T], F32, tag="s", name="ps4t0")
                        for p in range(npair - 1):
                            ph4_mm(pss0, 0, p)
                        while pending_sc:
                            pending_sc.pop()()
                        while pending_pe:
                            pending_pe.pop()()
                        ph4_mm(pss0, 0, npair - 1)
                        ph4_tail(pss0, 0)
                        for jo in range(1, nH):
                            pss = psS.tile([P, T], F32, tag="s",
                                           name=f"ps4t{jo}")
                            for p in range(npair):
                                ph4_mm(pss, jo, p)
                            ph4_tail(pss, jo)

    nc.compile()
    return nc


def prep_inputs(inputs, T, H, heads):
    """Host-side prep: returns list of per-core in_map dicts."""
    hd = H // heads
    nH = H // P
    npair = heads // (P // hd)
    scale = hd ** -0.5
    B = inputs["inputs"].shape[1]
    bf = ml_dtypes.bfloat16

    x = np.asarray(inputs["inputs"], np.float32)          # [T, B, H]
    pos = np.asarray(inputs["pos"], np.float32)[:, 0, :]  # [Lr, H]
    Win = np.asarray(inputs["input_weights"], np.float32)  # [3H, H]
    bin_ = np.asarray(inputs["input_biases"], np.float32)  # [3H]
    Wp = np.asarray(inputs["pos_weights"], np.float32)     # [H, H]
    bp = np.asarray(inputs["pos_biases"], np.float32)      # [H]
    Wo = np.asarray(inputs["output_weights"], np.float32)  # [H, H]
    bo = np.asarray(inputs["output_biases"], np.float32)   # [H]
    r_i = np.asarray(inputs["r_i"], np.float32)
    s_i = np.asarray(inputs["s_i"], np.float32)
    r_p = np.asarray(inputs["r_p"], np.float32)
    s_p = np.asarray(inputs["s_p"], np.float32)
    rw = np.asarray(inputs["r_w_bias"], np.float32)        # [heads, hd]
    rr = np.asarray(inputs["r_r_bias"], np.float32)        # [heads, hd]

    posT = np.ascontiguousarray(pos.T).astype(bf)          # [H, Lr]
    ident = np.eye(P, dtype=bf)

    b3 = bin_.reshape(heads, 3, hd)
    bq = ((b3[:, 0, :] + rw) * scale).reshape(H)
    bk = b3[:, 1, :].reshape(H)
    bv = b3[:, 2, :].reshape(H)
    drr = (scale * (rr - rw)).reshape(H)
    bo_eff = bo + Wo @ bv

    def tile_kb(W):  # [H(in), H(out)] -> [nH, P, H] (kb-major rows)
        return np.ascontiguousarray(W.reshape(nH, P, H)).astype(bf)

    def tile_bias(v):  # [H] -> [nH, P, 1]
        return np.ascontiguousarray(v.reshape(nH, P, 1))

    WoT = np.ascontiguousarray(Wo.T)  # [H(in: head*hd), H(out)]
    wo_t = np.ascontiguousarray(WoT.reshape(npair, P, H)).astype(bf)

    in_maps = []
    for b in range(B):
        WeffT = (Win.T * r_i[b][:, None]) * s_i[b][None, :]   # [H, 3H]
        We = WeffT.reshape(H, heads, 3, hd)
        WqT = np.ascontiguousarray(We[:, :, 0, :].reshape(H, H) * scale)
        WkT = np.ascontiguousarray(We[:, :, 1, :].reshape(H, H))
        WvT = np.ascontiguousarray(We[:, :, 2, :].reshape(H, H))
        WpT = (Wp.T * r_p[b][:, None]) * s_p[b][None, :]      # [H, H]
        in_maps.append({
            "xT": np.ascontiguousarray(x[:, b, :].T).astype(bf),
            "posT": posT,
            "wq": tile_kb(WqT),
            "wk": tile_kb(WkT),
            "wp": tile_kb(np.ascontiguousarray(WpT)),
            "wv": tile_kb(WvT),
            "wo": wo_t,
            "bqrw": tile_bias(bq),
            "bk": tile_bias(bk),
            "bp": tile_bias(bp),
            "drr": tile_bias(drr),
            "bo": tile_bias(bo_eff),
            "ident": ident,
        })
    return in_maps


_CACHE = {}
LAST_RESULT = None


def _get_program(T, H, heads, num_devices):
    key = (T, H, heads, num_devices)
    if key not in _CACHE:
        _CACHE[key] = build_program(T, H, heads, num_devices=num_devices)
    return _CACHE[key]


def kernel(**inputs):
    global LAST_RESULT
    T, B, H = inputs["inputs"].shape
    heads = int(inputs["heads"])
    nc = _get_program(T, H, heads, num_devices=B)
    in_maps = prep_inputs(inputs, T, H, heads)
    res = run_bass_kernel_spmd(nc, in_maps, core_ids=list(range(B)))
    LAST_RESULT = res
    out = np.stack([res.results[b]["outT"].T for b in range(B)], axis=1)
    return np.ascontiguousarray(out.astype(np.float32))


def run_profiled(**inputs):
    """Like kernel() but with trace=True; returns (out, BassKernelResults)."""
    global LAST_RESULT
    T, B, H = inputs["inputs"].shape
    heads = int(inputs["heads"])
    nc = _get_program(T, H, heads, num_devices=B)
    in_maps = prep_inputs(inputs, T, H, heads)
    res = run_bass_kernel_spmd(nc, in_maps, core_ids=list(range(B)), trace=True)
    LAST_RESULT = res
    out = np.stack([res.results[b]["outT"].T for b in range(B)], axis=1)
    return np.ascontiguousarray(out.astype(np.float32)), res


if __name__ == "__main__":
    # quick build check
    build_program()
    print("build OK")
